# revision 4
# baseline (speedup 1.0000x reference)
"""MoE-LoRA fused attention kernel for 8 Trainium2 NeuronCores.

Problem: x[8,512,768] -> qkv = x@W_qkv.T + top2-routed LoRA experts;
multi-head attention (12 heads, hd=64); out-projection.

Sharding: data-parallel over batch. Core b handles batch element b
(attention + routing are token-local, so there is no cross-core
communication at all).

Per-core layout strategy (everything host-pre-transposed so the device
does no transposes on the forward path):
  xT      [768, 512]   (d on partitions, t free)
  q,k     computed transposed:  qkT[o, t] = sum_d W[o,d] xT[d,t]
  v       computed natural:     v[t, o]
  scores  computed transposed:  st[kt, q] = kT.T @ qT  (exp is elementwise;
          the softmax normalizer Z[q] = sum_k exp(st) falls out of the
          O-matmul as a ones-column appended to v)
  O       computed natural:     O[q, hd|Z] = st_exp.T @ [v | 1]
  proj    needs attn_out transposed -> 24 PE transposes, then
          final[t, o] = attn_outT.T @ W_projT
Matmuls run as float32r (full PE rate at N>=256); the attention
O-matmul (N=65) runs in bf16.
"""

import os
import sys
import types

import numpy as np

for _p in ("/opt/trn_rl_repo",):
    if _p not in sys.path and os.path.isdir(_p):
        sys.path.append(_p)

import concourse.bass as bass  # noqa: E402
import concourse.tile as tile  # noqa: E402
from concourse import bacc, mybir  # noqa: E402
from concourse.bass import ts  # noqa: E402
from concourse.bass_utils import run_bass_kernel_spmd  # noqa: E402
from concourse.masks import make_identity  # noqa: E402

# ---- problem constants (hardcoded per contract) ----
B_SZ, S, D = 8, 512, 768
H = 12
N_EXP = 8
RANK = 16
ALPHA = 32
TOP_K = 2
HD = D // H            # 64
T = S                  # tokens per core
NR = N_EXP * RANK      # 128
O3 = 3 * D             # 2304
N_CORES = 8

F32 = mybir.dt.float32
F32R = mybir.dt.float32r
BF16 = mybir.dt.bfloat16

DC = D // 128          # 6 d-chunks
TC = T // 128          # 4 token-chunks
QKC = (2 * D) // 128   # 12 o-chunks for q,k


def build_nc():
    """Build the SPMD program (same on all 8 cores)."""
    nc = bacc.Bacc("TRN2", target_bir_lowering=False, debug=False,
                   num_devices=N_CORES)

    dp = nc.declare_dram_parameter
    xT_d = dp("xT", [D, T], F32R, isOutput=False).ap()
    wqk_d = dp("wqkT", [D, 2 * D], F32R, isOutput=False).ap()   # q pre-scaled
    wv_d = dp("wvT", [D, D], F32R, isOutput=False).ap()
    wg_d = dp("wgT", [D, N_EXP], F32R, isOutput=False).ap()
    at_d = dp("aT", [D, NR], F32R, isOutput=False).ap()         # pre-scaled by alpha/r
    btqk_d = dp("btqk", [NR, 2 * D], F32R, isOutput=False).ap()  # q pre-scaled
    bv_d = dp("bv", [NR, D], F32R, isOutput=False).ap()
    wp_d = dp("wpT", [D, D], F32R, isOutput=False).ap()
    bqk_d = dp("bqk", [128, QKC], F32, isOutput=False).ap()    # col o = b_qkv chunk
    bvv_d = dp("bvv", [1, D], F32, isOutput=False).ap()
    bg_d = dp("bg", [1, N_EXP], F32, isOutput=False).ap()
    bp_d = dp("bp", [1, D], F32, isOutput=False).ap()
    e8_d = dp("e8", [N_EXP, NR], F32R, isOutput=False).ap()     # expert->slot expand
    out_d = dp("out", [T, D], F32, isOutput=True).ap()

    with tile.TileContext(nc) as tc:
        _body(nc, tc, xT_d, wqk_d, wv_d, wg_d, at_d, btqk_d, bv_d, wp_d,
              bqk_d, bvv_d, bg_d, bp_d, e8_d, out_d)
    nc.compile()
    return nc


def _body(nc, tc, xT_d, wqk_d, wv_d, wg_d, at_d, btqk_d, bv_d, wp_d,
          bqk_d, bvv_d, bg_d, bp_d, e8_d, out_d):
    from contextlib import ExitStack
    ctx = ExitStack()
    with ctx:
        singles = ctx.enter_context(tc.tile_pool(name="singles", bufs=1))
        acts = ctx.enter_context(tc.tile_pool(name="acts", bufs=1))
        stx = ctx.enter_context(tc.tile_pool(name="stx", bufs=8))
        small_sb = ctx.enter_context(tc.tile_pool(name="small_sb", bufs=4))
        ps512 = ctx.enter_context(tc.tile_pool(name="ps512", bufs=2, space="PSUM"))
        ps768 = ctx.enter_context(tc.tile_pool(name="ps768", bufs=2, space="PSUM"))
        ps_small = ctx.enter_context(tc.tile_pool(name="ps_small", bufs=2, space="PSUM"))

        # ---- load everything ----
        def load(pool, shape, src, name, dt=F32):
            t = pool.tile(shape, dt, tag=name, name=name)
            nc.sync.dma_start(out=t, in_=src)
            return t

        xT = [load(singles, [128, T], xT_d[ts(k, 128), :], f"xT{k}", F32R) for k in range(DC)]
        wqk = [load(singles, [128, 2 * D], wqk_d[ts(k, 128), :], f"wqk{k}", F32R) for k in range(DC)]
        wv = [load(singles, [128, D], wv_d[ts(k, 128), :], f"wv{k}", F32R) for k in range(DC)]
        wg = [load(singles, [128, N_EXP], wg_d[ts(k, 128), :], f"wg{k}", F32R) for k in range(DC)]
        aT = [load(singles, [128, NR], at_d[ts(k, 128), :], f"aT{k}", F32R) for k in range(DC)]
        wp = [load(singles, [128, D], wp_d[ts(k, 128), :], f"wp{k}", F32R) for k in range(DC)]
        btqk = load(singles, [NR, 2 * D], btqk_d, "btqk", F32R)
        bv = load(singles, [NR, D], bv_d, "bv", F32R)
        bqk = load(singles, [128, QKC], bqk_d, "bqk")
        bvv = load(singles, [1, D], bvv_d, "bvv")
        bg = load(singles, [1, N_EXP], bg_d, "bg")
        bp = load(singles, [1, D], bp_d, "bp")
        e8 = load(singles, [N_EXP, NR], e8_d, "e8", F32R)

        ident = singles.tile([128, 128], F32, tag="ident")
        make_identity(nc, ident)
        ones_row = singles.tile([1, 128], F32, tag="ones_row")
        nc.vector.memset(ones_row, 1.0)

        # ---- router: probs -> top2 dispatch [t, 8], then expand to [128 nr, t] ----
        dispT = acts.tile([N_EXP, T], F32R, tag="dispT")
        for tci in range(TC):
            lg = ps_small.tile([128, N_EXP], F32, tag="ps_sm")
            for k in range(DC):
                nc.tensor.matmul(lg, (xT[k][:, ts(tci, 128)]),
                                 (wg[k]), start=(k == 0), stop=False)
            nc.tensor.matmul(lg, ones_row, bg, start=False, stop=True)
            probs = small_sb.tile([128, N_EXP], F32, tag="probs")
            sums = small_sb.tile([128, 1], F32, tag="sums")
            nc.scalar.activation(probs, lg, mybir.ActivationFunctionType.Exp,
                                 accum_out=sums)
            recip = small_sb.tile([128, 1], F32, tag="recip")
            nc.vector.reciprocal(recip, sums)
            nc.vector.tensor_scalar_mul(probs, probs, recip)
            m1 = small_sb.tile([128, 1], F32, tag="m1")
            nc.vector.reduce_max(m1, probs, axis=mybir.AxisListType.X)
            masked = small_sb.tile([128, N_EXP], F32, tag="masked")
            nc.vector.tensor_scalar(masked, probs, m1, None,
                                    op0=mybir.AluOpType.is_equal)
            nc.vector.tensor_scalar_mul(masked, masked, -10.0)
            nc.vector.tensor_add(masked, masked, probs)
            m2 = small_sb.tile([128, 1], F32, tag="m2")
            nc.vector.reduce_max(m2, masked, axis=mybir.AxisListType.X)
            ge = small_sb.tile([128, N_EXP], F32, tag="ge")
            nc.vector.tensor_scalar(ge, probs, m2, None,
                                    op0=mybir.AluOpType.is_ge)
            disp = small_sb.tile([128, N_EXP], F32, tag="disp")
            nc.vector.tensor_mul(disp, probs, ge)
            # transpose [128, 8] -> [8, 128] and collect into dispT
            trp = ps_small.tile([N_EXP, 128], F32, tag="ps_sm")
            nc.tensor.transpose(trp, disp, ident)
            nc.vector.tensor_copy(dispT[:, ts(tci, 128)], trp)

        # a_downT[nr, t] = (scaled A_flat) @ x  ;  cT = a_downT * expand(dispT)
        adn = ps512.tile([128, T], F32, tag="ps512")
        for k in range(DC):
            nc.tensor.matmul(adn, (aT[k]), (xT[k]),
                             start=(k == 0), stop=(k == DC - 1))
        adn_sb = acts.tile([128, T], F32, tag="adn_sb")
        nc.scalar.activation(adn_sb, adn, mybir.ActivationFunctionType.Copy)
        expd = ps512.tile([128, T], F32, tag="ps512")
        nc.tensor.matmul(expd, (e8), (dispT), start=True, stop=True)
        cT = acts.tile([128, T], F32R, tag="cT")
        nc.vector.tensor_mul(cT, adn_sb, expd)

        # ---- q,k transposed: qkT[oc][o 128, t 512], o chunks 0..11 ----
        qkT = []
        for oc in range(QKC):
            pq = ps512.tile([128, T], F32, tag="ps512")
            for k in range(DC):
                nc.tensor.matmul(pq, (wqk[k][:, ts(oc, 128)]), (xT[k]),
                                 start=(k == 0), stop=False)
            nc.tensor.matmul(pq, (btqk[:, ts(oc, 128)]), (cT),
                             start=False, stop=True)
            sb = acts.tile([128, T], F32R, tag=f"qkT{oc}")
            nc.scalar.activation(sb, pq, mybir.ActivationFunctionType.Identity,
                                 bias=bqk[:, oc:oc + 1])
            qkT.append(sb)

        # ---- v natural + ones column: v_aug[tc][128, 12*65] bf16 ----
        v_aug = []
        for tci in range(TC):
            pv = ps768.tile([128, D], F32, tag="ps768")
            for (n0, nsz) in ((0, 512), (512, 256)):
                for k in range(DC):
                    nc.tensor.matmul(pv[:, n0:n0 + nsz],
                                     (xT[k][:, ts(tci, 128)]),
                                     (wv[k][:, n0:n0 + nsz]),
                                     start=(k == 0), stop=False)
                nc.tensor.matmul(pv[:, n0:n0 + nsz], (cT[:, ts(tci, 128)]),
                                 (bv[:, n0:n0 + nsz]), start=False, stop=False)
                nc.tensor.matmul(pv[:, n0:n0 + nsz], ones_row,
                                 bvv[:, n0:n0 + nsz], start=False, stop=True)
            va = acts.tile([128, H, HD + 1], BF16, tag=f"v_aug{tci}")
            nc.vector.tensor_copy(va[:, :, 0:HD],
                                  pv.rearrange("p (h c) -> p h c", c=HD))
            nc.vector.memset(va[:, :, HD], 1.0)
            v_aug.append(va)

        # ---- attention ----
        attn_out = [acts.tile([128, D], F32, tag=f"attn_out{tci}", name=f"attn_out{tci}")
                    for tci in range(TC)]
        for h in range(H):
            qt = qkT[h // 2]
            kt = qkT[6 + h // 2]
            po = (h % 2) * HD
            st_exp = []
            for kc in range(TC):
                pst = ps512.tile([128, T], F32, tag="ps512")
                nc.tensor.matmul(pst, (kt[po:po + HD, ts(kc, 128)]),
                                 (qt[po:po + HD, :]), start=True, stop=True)
                se = stx.tile([128, T], BF16, tag="st_exp")
                nc.scalar.activation(se, pst, mybir.ActivationFunctionType.Exp)
                st_exp.append(se)
            for qc in range(TC):
                po2 = ps_small.tile([128, HD + 1], F32, tag="ps_sm")
                for kc in range(TC):
                    nc.tensor.matmul(po2, st_exp[kc][:, ts(qc, 128)],
                                     v_aug[kc][:, h, :],
                                     start=(kc == 0), stop=(kc == TC - 1))
                rz = small_sb.tile([128, 1], F32, tag="rz")
                nc.vector.reciprocal(rz, po2[:, HD:HD + 1])
                nc.vector.tensor_scalar_mul(attn_out[qc][:, ts(h, HD)],
                                            po2[:, 0:HD], rz)

        # ---- transpose attn_out -> attn_outT [d 128, t 512] x6 ----
        aoT = [acts.tile([128, T], F32R, tag=f"aoT{dc}", name=f"aoT{dc}") for dc in range(DC)]
        for tci in range(TC):
            for dc in range(DC):
                ptr = ps_small.tile([128, 128], F32, tag="ps_sm")
                nc.tensor.transpose(ptr, attn_out[tci][:, ts(dc, 128)], ident)
                nc.vector.tensor_copy(aoT[dc][:, ts(tci, 128)], ptr)

        # ---- final projection ----
        for tci in range(TC):
            pf = ps768.tile([128, D], F32, tag="ps768")
            for (n0, nsz) in ((0, 512), (512, 256)):
                for dc in range(DC):
                    nc.tensor.matmul(pf[:, n0:n0 + nsz],
                                     (aoT[dc][:, ts(tci, 128)]),
                                     (wp[dc][:, n0:n0 + nsz]),
                                     start=(dc == 0), stop=False)
                nc.tensor.matmul(pf[:, n0:n0 + nsz], ones_row,
                                 bp[:, n0:n0 + nsz], start=False, stop=True)
            osb = acts.tile([128, D], F32, tag=f"out_sb{tci}")
            nc.scalar.activation(osb, pf, mybir.ActivationFunctionType.Copy)
            nc.sync.dma_start(out=out_d[ts(tci, 128), :], in_=osb)


def prep_inputs(x, W_qkv, b_qkv, W_gate, b_gate, A, B_lora, W_proj, b_proj):
    """Host-side prep: pre-transpose/pre-scale weights, shard x by batch."""
    scale = HD ** -0.5
    scaling = ALPHA / RANK
    W_qkv = np.asarray(W_qkv, np.float32).copy()
    b_qkv = np.asarray(b_qkv, np.float32).copy()
    B_lora = np.asarray(B_lora, np.float32).copy()
    W_qkv[:D] *= scale          # fold attention scale into q
    b_qkv[:D] *= scale
    B_lora[:, :D, :] *= scale

    wqkT = np.ascontiguousarray(W_qkv[:2 * D].T)                # [768, 1536]
    wvT = np.ascontiguousarray(W_qkv[2 * D:].T)                 # [768, 768]
    wgT = np.ascontiguousarray(np.asarray(W_gate, np.float32).T)  # [768, 8]
    aT = np.ascontiguousarray(
        (np.asarray(A, np.float32).reshape(NR, D) * scaling).T)  # [768, 128]
    bt = np.ascontiguousarray(
        B_lora.transpose(0, 2, 1).reshape(NR, O3))               # [128, 2304]
    btqk = np.ascontiguousarray(bt[:, :2 * D])
    bvm = np.ascontiguousarray(bt[:, 2 * D:])
    wpT = np.ascontiguousarray(np.asarray(W_proj, np.float32).T)
    bqk = np.ascontiguousarray(b_qkv[:2 * D].reshape(QKC, 128).T)  # [128, 12]
    bvv = np.ascontiguousarray(b_qkv[2 * D:].reshape(1, D))
    bg = np.ascontiguousarray(np.asarray(b_gate, np.float32).reshape(1, N_EXP))
    bp = np.ascontiguousarray(np.asarray(b_proj, np.float32).reshape(1, D))
    e8 = np.ascontiguousarray(np.repeat(np.eye(N_EXP, dtype=np.float32), RANK, axis=1))

    shared = dict(wqkT=wqkT, wvT=wvT, wgT=wgT, aT=aT, btqk=btqk, bv=bvm,
                  wpT=wpT, bqk=bqk, bvv=bvv, bg=bg, bp=bp, e8=e8)
    x = np.asarray(x, np.float32)
    in_maps = []
    for b in range(N_CORES):
        m = dict(shared)
        m["xT"] = np.ascontiguousarray(x[b].T)
        in_maps.append(m)
    return in_maps


def _install_ntff_shim():
    """run_bass_kernel_spmd(trace=True) under axon needs antenv.axon_hooks."""
    if "antenv.axon_hooks" in sys.modules:
        return
    try:
        from trn_agent_boot.trn_boot import _ntff_profile_via_ctypes
        hook = _ntff_profile_via_ctypes("/opt/axon/libaxon_pjrt.so")
    except Exception:
        hook = None
    mod = types.ModuleType("antenv.axon_hooks")
    mod.get_axon_ntff_profile_hook = lambda: hook
    mod.set_axon_ntff_profile_hook = lambda h: None
    sys.modules["antenv.axon_hooks"] = mod


_NC_CACHE = None


def kernel(x, W_qkv, b_qkv, W_gate, b_gate, A, B_lora, W_proj, b_proj,
           _trace=False):
    global _NC_CACHE
    if _NC_CACHE is None:
        _NC_CACHE = build_nc()
    nc = _NC_CACHE
    in_maps = prep_inputs(x, W_qkv, b_qkv, W_gate, b_gate, A, B_lora,
                          W_proj, b_proj)
    if _trace:
        _install_ntff_shim()
    res = run_bass_kernel_spmd(nc, in_maps, list(range(N_CORES)), trace=_trace)
    out = np.stack([res.results[i]["out"] for i in range(N_CORES)], axis=0)
    out = out.reshape(B_SZ, S, D)
    if _trace:
        kernel.last_exec_time_ns = res.exec_time_ns
        kernel.last_results = res
    return out


# revision 5
# speedup vs baseline: 1.1699x; 1.1699x over previous
"""MoE-LoRA fused attention kernel for 8 Trainium2 NeuronCores.

Problem: x[8,512,768] -> qkv = x@W_qkv.T + top2-routed LoRA experts;
multi-head attention (12 heads, hd=64); out-projection.

Sharding: data-parallel over batch. Core b handles batch element b
(attention + routing are token-local, so there is no cross-core
communication at all).

Per-core layout strategy (everything host-pre-transposed so the device
does no transposes on the forward path):
  xT      [768, 512]   (d on partitions, t free)
  q,k     computed transposed:  qkT[o, t] = sum_d W[o,d] xT[d,t]
  v       computed natural:     v[t, o]
  scores  computed transposed:  st[kt, q] = kT.T @ qT  (exp is elementwise;
          the softmax normalizer Z[q] = sum_k exp(st) falls out of the
          O-matmul as a ones-column appended to v)
  O       computed natural:     O[q, hd|Z] = st_exp.T @ [v | 1]
  proj    needs attn_out transposed -> 24 PE transposes, then
          final[t, o] = attn_outT.T @ W_projT
Matmuls run as float32r (full PE rate at N>=256); the attention
O-matmul (N=65) runs in bf16.
"""

import os
import sys
import types

import numpy as np

for _p in ("/opt/trn_rl_repo",):
    if _p not in sys.path and os.path.isdir(_p):
        sys.path.append(_p)

import concourse.bass as bass  # noqa: E402
import concourse.tile as tile  # noqa: E402
from concourse import bacc, mybir  # noqa: E402
from concourse.bass import ts  # noqa: E402
from concourse.bass_utils import run_bass_kernel_spmd  # noqa: E402
from concourse.masks import make_identity  # noqa: E402

# ---- problem constants (hardcoded per contract) ----
B_SZ, S, D = 8, 512, 768
H = 12
N_EXP = 8
RANK = 16
ALPHA = 32
TOP_K = 2
HD = D // H            # 64
T = S                  # tokens per core
NR = N_EXP * RANK      # 128
O3 = 3 * D             # 2304
N_CORES = 8

F32 = mybir.dt.float32
F32R = mybir.dt.float32r
BF16 = mybir.dt.bfloat16

DC = D // 128          # 6 d-chunks
TC = T // 128          # 4 token-chunks
QKC = (2 * D) // 128   # 12 o-chunks for q,k


def build_nc():
    """Build the SPMD program (same on all 8 cores)."""
    nc = bacc.Bacc("TRN2", target_bir_lowering=False, debug=False,
                   num_devices=N_CORES)

    dp = nc.declare_dram_parameter
    xT_d = dp("xT", [D, T], F32R, isOutput=False).ap()
    wqk_d = dp("wqkT", [D, 2 * D], F32R, isOutput=False).ap()   # q pre-scaled
    wv_d = dp("wvT", [D, D], F32R, isOutput=False).ap()
    wg_d = dp("wgT", [D, N_EXP], F32R, isOutput=False).ap()
    at_d = dp("aT", [D, NR], F32R, isOutput=False).ap()         # pre-scaled by alpha/r
    btqk_d = dp("btqk", [NR, 2 * D], F32R, isOutput=False).ap()  # q pre-scaled
    bv_d = dp("bv", [NR, D], F32R, isOutput=False).ap()
    wp_d = dp("wpT", [D, D], F32R, isOutput=False).ap()
    bqk_d = dp("bqk", [128, QKC], F32, isOutput=False).ap()    # col o = b_qkv chunk
    bvv_d = dp("bvv", [1, D], F32, isOutput=False).ap()
    bg_d = dp("bg", [1, N_EXP], F32, isOutput=False).ap()
    bp_d = dp("bp", [1, D], F32, isOutput=False).ap()
    e8_d = dp("e8", [N_EXP, NR], F32R, isOutput=False).ap()     # expert->slot expand
    out_d = dp("out", [T, D], F32, isOutput=True).ap()

    with tile.TileContext(nc) as tc:
        _body(nc, tc, xT_d, wqk_d, wv_d, wg_d, at_d, btqk_d, bv_d, wp_d,
              bqk_d, bvv_d, bg_d, bp_d, e8_d, out_d)
    nc.compile()
    return nc


def _body(nc, tc, xT_d, wqk_d, wv_d, wg_d, at_d, btqk_d, bv_d, wp_d,
          bqk_d, bvv_d, bg_d, bp_d, e8_d, out_d):
    from contextlib import ExitStack
    ctx = ExitStack()
    with ctx:
        singles = ctx.enter_context(tc.tile_pool(name="singles", bufs=1))
        acts = ctx.enter_context(tc.tile_pool(name="acts", bufs=1))
        stx = ctx.enter_context(tc.tile_pool(name="stx", bufs=8))
        small_sb = ctx.enter_context(tc.tile_pool(name="small_sb", bufs=4))
        ps512 = ctx.enter_context(tc.tile_pool(name="ps512", bufs=2, space="PSUM"))
        ps768 = ctx.enter_context(tc.tile_pool(name="ps768", bufs=2, space="PSUM"))
        ps_small = ctx.enter_context(tc.tile_pool(name="ps_small", bufs=2, space="PSUM"))

        # ---- load everything ----
        def load(pool, shape, src, name, dt=F32):
            t = pool.tile(shape, dt, tag=name, name=name)
            nc.sync.dma_start(out=t, in_=src)
            return t

        # order matters: the DMA queues drain in submit order, so put the
        # small tensors the router/lora prologue needs ahead of the big
        # weight matrices.
        xT = [load(singles, [128, T], xT_d[ts(k, 128), :], f"xT{k}", F32R) for k in range(DC)]
        wg = [load(singles, [128, N_EXP], wg_d[ts(k, 128), :], f"wg{k}", F32R) for k in range(DC)]
        aT = [load(singles, [128, NR], at_d[ts(k, 128), :], f"aT{k}", F32R) for k in range(DC)]
        e8 = load(singles, [N_EXP, NR], e8_d, "e8", F32R)
        bg = load(singles, [1, N_EXP], bg_d, "bg")
        bqk = load(singles, [128, QKC], bqk_d, "bqk")
        bvv = load(singles, [1, D], bvv_d, "bvv")
        bp = load(singles, [1, D], bp_d, "bp")
        btqk = load(singles, [NR, 2 * D], btqk_d, "btqk", F32R)
        bv = load(singles, [NR, D], bv_d, "bv", F32R)
        wqk = [load(singles, [128, 2 * D], wqk_d[ts(k, 128), :], f"wqk{k}", F32R) for k in range(DC)]
        wv = [load(singles, [128, D], wv_d[ts(k, 128), :], f"wv{k}", F32R) for k in range(DC)]
        wp = [load(singles, [128, D], wp_d[ts(k, 128), :], f"wp{k}", F32R) for k in range(DC)]

        ident = singles.tile([128, 128], F32, tag="ident")
        make_identity(nc, ident)
        ones_row = singles.tile([1, 128], F32, tag="ones_row")
        nc.vector.memset(ones_row, 1.0)

        # ---- router: probs -> top2 dispatch [t, 8], then expand to [128 nr, t] ----
        dispT = acts.tile([N_EXP, T], F32R, tag="dispT")
        for tci in range(TC):
            lg = ps_small.tile([128, N_EXP], F32, tag="ps_sm")
            for k in range(DC):
                nc.tensor.matmul(lg, (xT[k][:, ts(tci, 128)]),
                                 (wg[k]), start=(k == 0), stop=False)
            nc.tensor.matmul(lg, ones_row, bg, start=False, stop=True)
            probs = small_sb.tile([128, N_EXP], F32, tag="probs")
            sums = small_sb.tile([128, 1], F32, tag="sums")
            nc.scalar.activation(probs, lg, mybir.ActivationFunctionType.Exp,
                                 accum_out=sums)
            recip = small_sb.tile([128, 1], F32, tag="recip")
            nc.vector.reciprocal(recip, sums)
            nc.vector.tensor_scalar_mul(probs, probs, recip)
            m1 = small_sb.tile([128, 1], F32, tag="m1")
            nc.vector.reduce_max(m1, probs, axis=mybir.AxisListType.X)
            masked = small_sb.tile([128, N_EXP], F32, tag="masked")
            nc.vector.tensor_scalar(masked, probs, m1, None,
                                    op0=mybir.AluOpType.is_equal)
            nc.vector.tensor_scalar_mul(masked, masked, -10.0)
            nc.vector.tensor_add(masked, masked, probs)
            m2 = small_sb.tile([128, 1], F32, tag="m2")
            nc.vector.reduce_max(m2, masked, axis=mybir.AxisListType.X)
            ge = small_sb.tile([128, N_EXP], F32, tag="ge")
            nc.vector.tensor_scalar(ge, probs, m2, None,
                                    op0=mybir.AluOpType.is_ge)
            disp = small_sb.tile([128, N_EXP], F32, tag="disp")
            nc.vector.tensor_mul(disp, probs, ge)
            # transpose [128, 8] -> [8, 128] and collect into dispT
            trp = ps_small.tile([N_EXP, 128], F32, tag="ps_sm")
            nc.tensor.transpose(trp, disp, ident)
            nc.vector.tensor_copy(dispT[:, ts(tci, 128)], trp)

        # a_downT[nr, t] = (scaled A_flat) @ x  ;  cT = a_downT * expand(dispT)
        adn = ps512.tile([128, T], F32, tag="ps512")
        for k in range(DC):
            nc.tensor.matmul(adn, (aT[k]), (xT[k]),
                             start=(k == 0), stop=(k == DC - 1))
        adn_sb = acts.tile([128, T], F32, tag="adn_sb")
        nc.scalar.activation(adn_sb, adn, mybir.ActivationFunctionType.Copy)
        expd = ps512.tile([128, T], F32, tag="ps512")
        nc.tensor.matmul(expd, (e8), (dispT), start=True, stop=True)
        cT = acts.tile([128, T], F32R, tag="cT")
        nc.vector.tensor_mul(cT, adn_sb, expd)

        # ---- q,k transposed: qkT[oc][o 128, t 512], o chunks 0..11 ----
        qkT = []
        for oc in range(QKC):
            pq = ps512.tile([128, T], F32, tag="ps512")
            for k in range(DC):
                nc.tensor.matmul(pq, (wqk[k][:, ts(oc, 128)]), (xT[k]),
                                 start=(k == 0), stop=False)
            nc.tensor.matmul(pq, (btqk[:, ts(oc, 128)]), (cT),
                             start=False, stop=True)
            sb = acts.tile([128, T], BF16, tag=f"qkT{oc}")
            nc.scalar.activation(sb, pq, mybir.ActivationFunctionType.Identity,
                                 bias=bqk[:, oc:oc + 1])
            qkT.append(sb)

        # ---- v natural + ones column: v_aug[tc][128, 12*65] bf16 ----
        v_aug = []
        for tci in range(TC):
            pv = ps768.tile([128, D], F32, tag="ps768")
            for (n0, nsz) in ((0, 512), (512, 256)):
                for k in range(DC):
                    nc.tensor.matmul(pv[:, n0:n0 + nsz],
                                     (xT[k][:, ts(tci, 128)]),
                                     (wv[k][:, n0:n0 + nsz]),
                                     start=(k == 0), stop=False)
                nc.tensor.matmul(pv[:, n0:n0 + nsz], (cT[:, ts(tci, 128)]),
                                 (bv[:, n0:n0 + nsz]), start=False, stop=False)
                nc.tensor.matmul(pv[:, n0:n0 + nsz], ones_row,
                                 bvv[:, n0:n0 + nsz], start=False, stop=True)
            va = acts.tile([128, H, HD + 1], BF16, tag=f"v_aug{tci}")
            nc.vector.tensor_copy(va[:, :, 0:HD],
                                  pv.rearrange("p (h c) -> p h c", c=HD))
            nc.vector.memset(va[:, :, HD], 1.0)
            v_aug.append(va)

        # ---- attention ----
        attn_out = [acts.tile([128, D], F32, tag=f"attn_out{tci}", name=f"attn_out{tci}")
                    for tci in range(TC)]
        for h in range(H):
            qt = qkT[h // 2]
            kt = qkT[6 + h // 2]
            po = (h % 2) * HD
            st_exp = []
            for kc in range(TC):
                pst = ps512.tile([128, T], F32, tag="ps512")
                nc.tensor.matmul(pst, (kt[po:po + HD, ts(kc, 128)]),
                                 (qt[po:po + HD, :]), start=True, stop=True)
                se = stx.tile([128, T], BF16, tag="st_exp")
                nc.scalar.activation(se, pst, mybir.ActivationFunctionType.Exp)
                st_exp.append(se)
            for qc in range(TC):
                po2 = ps_small.tile([128, HD + 1], F32, tag="ps_sm")
                for kc in range(TC):
                    nc.tensor.matmul(po2, st_exp[kc][:, ts(qc, 128)],
                                     v_aug[kc][:, h, :],
                                     start=(kc == 0), stop=(kc == TC - 1))
                rz = small_sb.tile([128, 1], F32, tag="rz")
                nc.vector.reciprocal(rz, po2[:, HD:HD + 1])
                nc.vector.tensor_scalar_mul(attn_out[qc][:, ts(h, HD)],
                                            po2[:, 0:HD], rz)

        # ---- transpose attn_out -> attn_outT [d 128, t 512] x6 ----
        aoT = [acts.tile([128, T], F32R, tag=f"aoT{dc}", name=f"aoT{dc}") for dc in range(DC)]
        for tci in range(TC):
            for dc in range(DC):
                ptr = ps_small.tile([128, 128], F32, tag="ps_sm")
                nc.tensor.transpose(ptr, attn_out[tci][:, ts(dc, 128)], ident)
                nc.vector.tensor_copy(aoT[dc][:, ts(tci, 128)], ptr)

        # ---- final projection ----
        for tci in range(TC):
            pf = ps768.tile([128, D], F32, tag="ps768")
            for (n0, nsz) in ((0, 512), (512, 256)):
                for dc in range(DC):
                    nc.tensor.matmul(pf[:, n0:n0 + nsz],
                                     (aoT[dc][:, ts(tci, 128)]),
                                     (wp[dc][:, n0:n0 + nsz]),
                                     start=(dc == 0), stop=False)
                nc.tensor.matmul(pf[:, n0:n0 + nsz], ones_row,
                                 bp[:, n0:n0 + nsz], start=False, stop=True)
            osb = acts.tile([128, D], F32, tag=f"out_sb{tci}")
            nc.scalar.activation(osb, pf, mybir.ActivationFunctionType.Copy)
            nc.sync.dma_start(out=out_d[ts(tci, 128), :], in_=osb)


def prep_inputs(x, W_qkv, b_qkv, W_gate, b_gate, A, B_lora, W_proj, b_proj):
    """Host-side prep: pre-transpose/pre-scale weights, shard x by batch."""
    scale = HD ** -0.5
    scaling = ALPHA / RANK
    W_qkv = np.asarray(W_qkv, np.float32).copy()
    b_qkv = np.asarray(b_qkv, np.float32).copy()
    B_lora = np.asarray(B_lora, np.float32).copy()
    W_qkv[:D] *= scale          # fold attention scale into q
    b_qkv[:D] *= scale
    B_lora[:, :D, :] *= scale

    wqkT = np.ascontiguousarray(W_qkv[:2 * D].T)                # [768, 1536]
    wvT = np.ascontiguousarray(W_qkv[2 * D:].T)                 # [768, 768]
    wgT = np.ascontiguousarray(np.asarray(W_gate, np.float32).T)  # [768, 8]
    aT = np.ascontiguousarray(
        (np.asarray(A, np.float32).reshape(NR, D) * scaling).T)  # [768, 128]
    bt = np.ascontiguousarray(
        B_lora.transpose(0, 2, 1).reshape(NR, O3))               # [128, 2304]
    btqk = np.ascontiguousarray(bt[:, :2 * D])
    bvm = np.ascontiguousarray(bt[:, 2 * D:])
    wpT = np.ascontiguousarray(np.asarray(W_proj, np.float32).T)
    bqk = np.ascontiguousarray(b_qkv[:2 * D].reshape(QKC, 128).T)  # [128, 12]
    bvv = np.ascontiguousarray(b_qkv[2 * D:].reshape(1, D))
    bg = np.ascontiguousarray(np.asarray(b_gate, np.float32).reshape(1, N_EXP))
    bp = np.ascontiguousarray(np.asarray(b_proj, np.float32).reshape(1, D))
    e8 = np.ascontiguousarray(np.repeat(np.eye(N_EXP, dtype=np.float32), RANK, axis=1))

    shared = dict(wqkT=wqkT, wvT=wvT, wgT=wgT, aT=aT, btqk=btqk, bv=bvm,
                  wpT=wpT, bqk=bqk, bvv=bvv, bg=bg, bp=bp, e8=e8)
    x = np.asarray(x, np.float32)
    in_maps = []
    for b in range(N_CORES):
        m = dict(shared)
        m["xT"] = np.ascontiguousarray(x[b].T)
        in_maps.append(m)
    return in_maps


def _install_ntff_shim():
    """run_bass_kernel_spmd(trace=True) under axon needs antenv.axon_hooks."""
    if "antenv.axon_hooks" in sys.modules:
        return
    try:
        from trn_agent_boot.trn_boot import _ntff_profile_via_ctypes
        hook = _ntff_profile_via_ctypes("/opt/axon/libaxon_pjrt.so")
    except Exception:
        hook = None
    mod = types.ModuleType("antenv.axon_hooks")
    mod.get_axon_ntff_profile_hook = lambda: hook
    mod.set_axon_ntff_profile_hook = lambda h: None
    sys.modules["antenv.axon_hooks"] = mod


_NC_CACHE = None


def kernel(x, W_qkv, b_qkv, W_gate, b_gate, A, B_lora, W_proj, b_proj,
           _trace=False):
    global _NC_CACHE
    if _NC_CACHE is None:
        _NC_CACHE = build_nc()
    nc = _NC_CACHE
    in_maps = prep_inputs(x, W_qkv, b_qkv, W_gate, b_gate, A, B_lora,
                          W_proj, b_proj)
    if _trace:
        _install_ntff_shim()
    res = run_bass_kernel_spmd(nc, in_maps, list(range(N_CORES)), trace=_trace)
    out = np.stack([res.results[i]["out"] for i in range(N_CORES)], axis=0)
    out = out.reshape(B_SZ, S, D)
    if _trace:
        kernel.last_exec_time_ns = res.exec_time_ns
        kernel.last_results = res
    return out


# revision 6
# speedup vs baseline: 1.4016x; 1.1981x over previous
"""MoE-LoRA fused attention kernel for 8 Trainium2 NeuronCores.

Problem: x[8,512,768] -> qkv = x@W_qkv.T + top2-routed LoRA experts;
multi-head attention (12 heads, hd=64); out-projection.

Sharding: data-parallel over batch. Core b handles batch element b
(attention + routing are token-local, so there is no cross-core
communication at all).

Per-core layout strategy (everything host-pre-transposed so the device
does no transposes on the forward path):
  xT      [768, 512]   (d on partitions, t free)
  q,k     computed transposed:  qkT[o, t] = sum_d W[o,d] xT[d,t]
  v       computed natural:     v[t, o]
  scores  computed transposed:  st[kt, q] = kT.T @ qT  (exp is elementwise;
          the softmax normalizer Z[q] = sum_k exp(st) falls out of the
          O-matmul as a ones-column appended to v)
  O       computed natural:     O[q, hd|Z] = st_exp.T @ [v | 1]
  proj    needs attn_out transposed -> 24 PE transposes, then
          final[t, o] = attn_outT.T @ W_projT
Matmuls run as float32r (full PE rate at N>=256); the attention
O-matmul (N=65) runs in bf16.
"""

import os
import sys
import types

import numpy as np

for _p in ("/opt/trn_rl_repo",):
    if _p not in sys.path and os.path.isdir(_p):
        sys.path.append(_p)

import concourse.bass as bass  # noqa: E402
import concourse.tile as tile  # noqa: E402
from concourse import bacc, mybir  # noqa: E402
from concourse.bass import ts  # noqa: E402
from concourse.bass_utils import run_bass_kernel_spmd  # noqa: E402
from concourse.masks import make_identity  # noqa: E402

# ---- problem constants (hardcoded per contract) ----
B_SZ, S, D = 8, 512, 768
H = 12
N_EXP = 8
RANK = 16
ALPHA = 32
TOP_K = 2
HD = D // H            # 64
T = S                  # tokens per core
NR = N_EXP * RANK      # 128
O3 = 3 * D             # 2304
N_CORES = 8

F32 = mybir.dt.float32
F32R = mybir.dt.float32r
BF16 = mybir.dt.bfloat16

DC = D // 128          # 6 d-chunks
TC = T // 128          # 4 token-chunks
QKC = (2 * D) // 128   # 12 o-chunks for q,k


def build_nc():
    """Build the SPMD program (same on all 8 cores)."""
    nc = bacc.Bacc("TRN2", target_bir_lowering=False, debug=False,
                   num_devices=N_CORES)

    dp = nc.declare_dram_parameter
    xT_d = dp("xT", [D, T], F32R, isOutput=False).ap()
    wqk_d = dp("wqkT", [D, 2 * D], F32R, isOutput=False).ap()   # q pre-scaled
    wv_d = dp("wvT", [D, D], F32R, isOutput=False).ap()
    wg_d = dp("wgT", [D, N_EXP], F32R, isOutput=False).ap()
    at_d = dp("aT", [D, NR], F32R, isOutput=False).ap()         # pre-scaled by alpha/r
    btqk_d = dp("btqk", [NR, 2 * D], F32R, isOutput=False).ap()  # q pre-scaled
    bv_d = dp("bv", [NR, D], F32R, isOutput=False).ap()
    wp_d = dp("wpT", [D, D], F32R, isOutput=False).ap()
    bqk_d = dp("bqk", [128, QKC], F32, isOutput=False).ap()    # col o = b_qkv chunk
    bvv_d = dp("bvv", [1, D], F32, isOutput=False).ap()
    bg_d = dp("bg", [1, N_EXP], F32, isOutput=False).ap()
    bp_d = dp("bp", [1, D], F32, isOutput=False).ap()
    e8_d = dp("e8", [N_EXP, NR], F32R, isOutput=False).ap()     # expert->slot expand
    out_d = dp("out", [T, D], F32, isOutput=True).ap()

    with tile.TileContext(nc) as tc:
        _body(nc, tc, xT_d, wqk_d, wv_d, wg_d, at_d, btqk_d, bv_d, wp_d,
              bqk_d, bvv_d, bg_d, bp_d, e8_d, out_d)
    nc.compile()
    return nc


def _body(nc, tc, xT_d, wqk_d, wv_d, wg_d, at_d, btqk_d, bv_d, wp_d,
          bqk_d, bvv_d, bg_d, bp_d, e8_d, out_d):
    from contextlib import ExitStack
    ctx = ExitStack()
    with ctx:
        singles = ctx.enter_context(tc.tile_pool(name="singles", bufs=1))
        acts = ctx.enter_context(tc.tile_pool(name="acts", bufs=1))
        stx = ctx.enter_context(tc.tile_pool(name="stx", bufs=8))
        small_sb = ctx.enter_context(tc.tile_pool(name="small_sb", bufs=4))
        ps512 = ctx.enter_context(tc.tile_pool(name="ps512", bufs=2, space="PSUM"))
        ps768 = ctx.enter_context(tc.tile_pool(name="ps768", bufs=2, space="PSUM"))
        ps_small = ctx.enter_context(tc.tile_pool(name="ps_small", bufs=2, space="PSUM"))

        # ---- load everything ----
        def load(pool, shape, src, name, dt=F32):
            t = pool.tile(shape, dt, tag=name, name=name)
            nc.sync.dma_start(out=t, in_=src)
            return t

        # order matters: the DMA queues drain in submit order, so put the
        # small tensors the router/lora prologue needs ahead of the big
        # weight matrices.
        xT = [load(singles, [128, T], xT_d[ts(k, 128), :], f"xT{k}", F32R) for k in range(DC)]
        wg = [load(singles, [128, N_EXP], wg_d[ts(k, 128), :], f"wg{k}", F32R) for k in range(DC)]
        aT = [load(singles, [128, NR], at_d[ts(k, 128), :], f"aT{k}", F32R) for k in range(DC)]
        e8 = load(singles, [N_EXP, NR], e8_d, "e8", F32R)
        bqk = load(singles, [128, QKC], bqk_d, "bqk")
        btqk = load(singles, [NR, 2 * D], btqk_d, "btqk", F32R)
        bv = load(singles, [NR, D], bv_d, "bv", F32R)
        wqk = [load(singles, [128, 2 * D], wqk_d[ts(k, 128), :], f"wqk{k}", F32R) for k in range(DC)]
        wv = [load(singles, [128, D], wv_d[ts(k, 128), :], f"wv{k}", F32R) for k in range(DC)]
        wp = [load(singles, [128, D], wp_d[ts(k, 128), :], f"wp{k}", F32R) for k in range(DC)]

        # biases broadcast to 128 partitions via DMA (DRE replicate) so no
        # K=1 matmuls are needed
        bg_full = singles.tile([128, N_EXP], F32, tag="bg_full")
        nc.gpsimd.dma_start(out=bg_full, in_=bg_d.partition_broadcast(128)[:, 0, :])
        bvv_full = singles.tile([128, D], F32, tag="bvv_full")
        nc.gpsimd.dma_start(out=bvv_full, in_=bvv_d.partition_broadcast(128)[:, 0, :])
        bp_full = singles.tile([128, D], F32, tag="bp_full")
        nc.gpsimd.dma_start(out=bp_full, in_=bp_d.partition_broadcast(128)[:, 0, :])

        ident = singles.tile([128, 128], F32, tag="ident")
        make_identity(nc, ident)
        ones_row = singles.tile([1, 128], F32, tag="ones_row")
        nc.vector.memset(ones_row, 1.0)

        # ---- router: probs -> top2 dispatch [t, 8], then expand to [128 nr, t] ----
        dispT = acts.tile([N_EXP, T], F32R, tag="dispT")
        for tci in range(TC):
            lg = ps_small.tile([128, N_EXP], F32, tag="ps_sm")
            for k in range(DC):
                nc.tensor.matmul(lg, (xT[k][:, ts(tci, 128)]),
                                 (wg[k]), start=(k == 0), stop=(k == DC - 1))
            lgb = small_sb.tile([128, N_EXP], F32, tag="lgb")
            nc.vector.tensor_add(lgb, lg, bg_full)
            probs = small_sb.tile([128, N_EXP], F32, tag="probs")
            sums = small_sb.tile([128, 1], F32, tag="sums")
            nc.scalar.activation(probs, lgb, mybir.ActivationFunctionType.Exp,
                                 accum_out=sums)
            recip = small_sb.tile([128, 1], F32, tag="recip")
            nc.vector.reciprocal(recip, sums)
            nc.vector.tensor_scalar_mul(probs, probs, recip)
            m1 = small_sb.tile([128, 1], F32, tag="m1")
            nc.vector.reduce_max(m1, probs, axis=mybir.AxisListType.X)
            masked = small_sb.tile([128, N_EXP], F32, tag="masked")
            nc.vector.tensor_scalar(masked, probs, m1, None,
                                    op0=mybir.AluOpType.is_equal)
            nc.vector.tensor_scalar_mul(masked, masked, -10.0)
            nc.vector.tensor_add(masked, masked, probs)
            m2 = small_sb.tile([128, 1], F32, tag="m2")
            nc.vector.reduce_max(m2, masked, axis=mybir.AxisListType.X)
            ge = small_sb.tile([128, N_EXP], F32, tag="ge")
            nc.vector.tensor_scalar(ge, probs, m2, None,
                                    op0=mybir.AluOpType.is_ge)
            disp = small_sb.tile([128, N_EXP], F32, tag="disp")
            nc.vector.tensor_mul(disp, probs, ge)
            # transpose [128, 8] -> [8, 128] and collect into dispT
            trp = ps_small.tile([N_EXP, 128], F32, tag="ps_sm")
            nc.tensor.transpose(trp, disp, ident)
            nc.vector.tensor_copy(dispT[:, ts(tci, 128)], trp)

        # a_downT[nr, t] = (scaled A_flat) @ x  ;  cT = a_downT * expand(dispT)
        adn = ps512.tile([128, T], F32, tag="ps512")
        for k in range(DC):
            nc.tensor.matmul(adn, (aT[k]), (xT[k]),
                             start=(k == 0), stop=(k == DC - 1))
        adn_sb = acts.tile([128, T], F32, tag="adn_sb")
        nc.scalar.activation(adn_sb, adn, mybir.ActivationFunctionType.Copy)
        expd = ps512.tile([128, T], F32, tag="ps512")
        nc.tensor.matmul(expd, (e8), (dispT), start=True, stop=True)
        cT = acts.tile([128, T], F32R, tag="cT")
        nc.vector.tensor_mul(cT, adn_sb, expd)

        # ---- q,k transposed: qkT[oc][o 128, t 512], o chunks 0..11 ----
        qkT = []
        for oc in range(QKC):
            pq = ps512.tile([128, T], F32, tag="ps512")
            for k in range(DC):
                nc.tensor.matmul(pq, (wqk[k][:, ts(oc, 128)]), (xT[k]),
                                 start=(k == 0), stop=False)
            nc.tensor.matmul(pq, (btqk[:, ts(oc, 128)]), (cT),
                             start=False, stop=True)
            sb = acts.tile([128, T], BF16, tag=f"qkT{oc}")
            nc.scalar.activation(sb, pq, mybir.ActivationFunctionType.Identity,
                                 bias=bqk[:, oc:oc + 1])
            qkT.append(sb)

        # ---- v natural + ones column: v_aug[tc][128, 12*65] bf16 ----
        v_aug = []
        for tci in range(TC):
            pv = ps768.tile([128, D], F32, tag="ps768")
            for (n0, nsz) in ((0, 512), (512, 256)):
                for k in range(DC):
                    nc.tensor.matmul(pv[:, n0:n0 + nsz],
                                     (xT[k][:, ts(tci, 128)]),
                                     (wv[k][:, n0:n0 + nsz]),
                                     start=(k == 0), stop=False)
                nc.tensor.matmul(pv[:, n0:n0 + nsz], (cT[:, ts(tci, 128)]),
                                 (bv[:, n0:n0 + nsz]), start=False, stop=True)
            va = acts.tile([128, H, HD + 1], BF16, tag=f"v_aug{tci}")
            nc.vector.tensor_add(va[:, :, 0:HD],
                                 pv.rearrange("p (h c) -> p h c", c=HD),
                                 bvv_full.rearrange("p (h c) -> p h c", c=HD))
            nc.vector.memset(va[:, :, HD], 1.0)
            v_aug.append(va)

        # ---- attention ----
        attn_out = [acts.tile([128, D], F32, tag=f"attn_out{tci}", name=f"attn_out{tci}")
                    for tci in range(TC)]
        for j in range(H // 2):
            # heads 2j (rows 0:64) and 2j+1 (rows 64:128) of qkT tile pair j:
            # emit their K=64 score matmuls back-to-back so the row-disjoint
            # pair packs onto the PE array concurrently.
            qt = qkT[j]
            kt = qkT[6 + j]
            st_exp = {0: [], 64: []}
            for kc in range(TC):
                for po in (0, 64):
                    pst = ps512.tile([128, T], F32, tag="ps512", name="pst")
                    nc.tensor.matmul(pst, (kt[po:po + HD, ts(kc, 128)]),
                                     (qt[po:po + HD, :]), start=True, stop=True,
                                     tile_position=(po, 0))
                    se = stx.tile([128, T], BF16, tag="st_exp", name="se")
                    nc.scalar.activation(se, pst,
                                         mybir.ActivationFunctionType.Exp)
                    st_exp[po].append(se)
            for qc in range(TC):
                for po in (0, 64):
                    h = 2 * j + po // 64
                    po2 = ps_small.tile([128, HD + 1], F32, tag="ps_sm",
                                        name="po2")
                    for kc in range(TC):
                        nc.tensor.matmul(po2, st_exp[po][kc][:, ts(qc, 128)],
                                         v_aug[kc][:, h, :],
                                         start=(kc == 0), stop=(kc == TC - 1))
                    rz = small_sb.tile([128, 1], F32, tag="rz", name="rz")
                    nc.vector.reciprocal(rz, po2[:, HD:HD + 1])
                    nc.vector.tensor_scalar_mul(attn_out[qc][:, ts(h, HD)],
                                                po2[:, 0:HD], rz)

        # ---- transpose attn_out -> attn_outT [d 128, t 512] x6 ----
        aoT = [acts.tile([128, T], F32R, tag=f"aoT{dc}", name=f"aoT{dc}") for dc in range(DC)]
        for tci in range(TC):
            for dc in range(DC):
                ptr = ps_small.tile([128, 128], F32, tag="ps_sm")
                nc.tensor.transpose(ptr, attn_out[tci][:, ts(dc, 128)], ident)
                nc.vector.tensor_copy(aoT[dc][:, ts(tci, 128)], ptr)

        # ---- final projection ----
        for tci in range(TC):
            pf = ps768.tile([128, D], F32, tag="ps768")
            for (n0, nsz) in ((0, 512), (512, 256)):
                for dc in range(DC):
                    nc.tensor.matmul(pf[:, n0:n0 + nsz],
                                     (aoT[dc][:, ts(tci, 128)]),
                                     (wp[dc][:, n0:n0 + nsz]),
                                     start=(dc == 0), stop=(dc == DC - 1))
            osb = acts.tile([128, D], F32, tag=f"out_sb{tci}")
            nc.vector.tensor_add(osb, pf, bp_full)
            nc.sync.dma_start(out=out_d[ts(tci, 128), :], in_=osb)


def prep_inputs(x, W_qkv, b_qkv, W_gate, b_gate, A, B_lora, W_proj, b_proj):
    """Host-side prep: pre-transpose/pre-scale weights, shard x by batch."""
    scale = HD ** -0.5
    scaling = ALPHA / RANK
    W_qkv = np.asarray(W_qkv, np.float32).copy()
    b_qkv = np.asarray(b_qkv, np.float32).copy()
    B_lora = np.asarray(B_lora, np.float32).copy()
    W_qkv[:D] *= scale          # fold attention scale into q
    b_qkv[:D] *= scale
    B_lora[:, :D, :] *= scale

    wqkT = np.ascontiguousarray(W_qkv[:2 * D].T)                # [768, 1536]
    wvT = np.ascontiguousarray(W_qkv[2 * D:].T)                 # [768, 768]
    wgT = np.ascontiguousarray(np.asarray(W_gate, np.float32).T)  # [768, 8]
    aT = np.ascontiguousarray(
        (np.asarray(A, np.float32).reshape(NR, D) * scaling).T)  # [768, 128]
    bt = np.ascontiguousarray(
        B_lora.transpose(0, 2, 1).reshape(NR, O3))               # [128, 2304]
    btqk = np.ascontiguousarray(bt[:, :2 * D])
    bvm = np.ascontiguousarray(bt[:, 2 * D:])
    wpT = np.ascontiguousarray(np.asarray(W_proj, np.float32).T)
    bqk = np.ascontiguousarray(b_qkv[:2 * D].reshape(QKC, 128).T)  # [128, 12]
    bvv = np.ascontiguousarray(b_qkv[2 * D:].reshape(1, D))
    bg = np.ascontiguousarray(np.asarray(b_gate, np.float32).reshape(1, N_EXP))
    bp = np.ascontiguousarray(np.asarray(b_proj, np.float32).reshape(1, D))
    e8 = np.ascontiguousarray(np.repeat(np.eye(N_EXP, dtype=np.float32), RANK, axis=1))

    shared = dict(wqkT=wqkT, wvT=wvT, wgT=wgT, aT=aT, btqk=btqk, bv=bvm,
                  wpT=wpT, bqk=bqk, bvv=bvv, bg=bg, bp=bp, e8=e8)
    x = np.asarray(x, np.float32)
    in_maps = []
    for b in range(N_CORES):
        m = dict(shared)
        m["xT"] = np.ascontiguousarray(x[b].T)
        in_maps.append(m)
    return in_maps


def _install_ntff_shim():
    """run_bass_kernel_spmd(trace=True) under axon needs antenv.axon_hooks."""
    if "antenv.axon_hooks" in sys.modules:
        return
    try:
        from trn_agent_boot.trn_boot import _ntff_profile_via_ctypes
        hook = _ntff_profile_via_ctypes("/opt/axon/libaxon_pjrt.so")
    except Exception:
        hook = None
    mod = types.ModuleType("antenv.axon_hooks")
    mod.get_axon_ntff_profile_hook = lambda: hook
    mod.set_axon_ntff_profile_hook = lambda h: None
    sys.modules["antenv.axon_hooks"] = mod


_NC_CACHE = None


def kernel(x, W_qkv, b_qkv, W_gate, b_gate, A, B_lora, W_proj, b_proj,
           _trace=False):
    global _NC_CACHE
    if _NC_CACHE is None:
        _NC_CACHE = build_nc()
    nc = _NC_CACHE
    in_maps = prep_inputs(x, W_qkv, b_qkv, W_gate, b_gate, A, B_lora,
                          W_proj, b_proj)
    if _trace:
        _install_ntff_shim()
    res = run_bass_kernel_spmd(nc, in_maps, list(range(N_CORES)), trace=_trace)
    out = np.stack([res.results[i]["out"] for i in range(N_CORES)], axis=0)
    out = out.reshape(B_SZ, S, D)
    if _trace:
        kernel.last_exec_time_ns = res.exec_time_ns
        kernel.last_results = res
    return out
